# revision 1
# baseline (speedup 1.0000x reference)
"""Self-contained Trainium2 Bass kernel for nn_Attention_26740466385723.

Full-input contract: kernel(**inputs) takes the unsharded numpy inputs and
returns the full [4, 1024, 1024] output.

Sharding (zero-collective): 8 cores = 4 batch x 2 sequence-halves. Each core
computes the attention output rows for half the sequence of one batch element.
The KV projection is duplicated within each batch pair (33% extra flops) which
avoids any inter-core collective.

v2: single fully-interleaved schedule. Engine queues execute in emission
order; the emission order software-pipelines all phases so the PE stream is
dense and the ACT exp stream starts ~25us into the kernel:

  PE:   q (k-major over 8 psum banks) | per c: kT(c) -> scores(pair c-1) ->
        v-block -> PV(pair c-2) -> selector+norm(pair c-3) | tail | out-proj
  ACT:  q/k psum->sbuf copies + the exp stream (the attention floor)
  DVE:  rotary (bf16 fast mode), v scatter copies, A^T copies, denom
        staging, approx-reciprocal, casts, normalize muls
  GPSIMD queue: xt own-half + cos/sin direct DMAs
  Scalar queue: xt other-half direct DMAs (done before the first ACT copy)
  Sync ring: sel/bo/ones + weights wq -> wk -> wv -> wo (async HWDGE)

Per-core device algorithm (layouts chosen so no on-device transpose is ever
needed):
  qT  = Wq^T x_own^T        [inner, 512]   (Wq stationary, xT moving)
  kT  = Wk^T x_all^T        [inner, 1024]
  v   = x_all Wv            [1024, inner]  (xT stationary, Wv moving)
  rotary on qT/kT: qrotT = qT*cos + shuffle(qT)*sin_signed (DVE lane swap)
  per head h: scoresT[j, r] = sum_d kT[d,j] qT[d,r]
              P^T = exp(scoresT * 0.125)   (no max subtraction; scores~N(0,1))
              pv  = [v_h | 1]^T @ P^T -> [65, r], row 64 = softmax denominator
  A^T normalized via two K=1 selector matmuls broadcasting 1/denom
  out = A Wo + bo          [512, 1024]    (A^T stationary, Wo moving)
"""

import sys
import os

if "/opt/trn_rl_repo" not in sys.path:
    sys.path.insert(0, "/opt/trn_rl_repo")

import numpy as np

HEADS = 16
DH = 64
B = 4
N = 1024
D = 1024
INNER = 1024
NCORES = 8
R = 512  # rows (query positions) per core
SCALE = DH ** -0.5  # 0.125

# matmul dtype: "bf16" (low power, FWL, 1 cyc/row), "f32r" (TF32-like,
# 1 cyc/row but power-throttles), or "f32" (exact, 4 cyc/row)
DTYPE_MODE = os.environ.get("BASS_ATTN_DTYPE", "bf16")

_CACHE = {}


def _build(dtype_mode: str, has_bias: bool = True):
    import concourse.bass as bass
    import concourse.mybir as mybir
    from concourse import bacc
    from concourse.tile import TileContext

    F32 = mybir.dt.float32
    MDT = {"bf16": mybir.dt.bfloat16,
           "f32r": mybir.dt.float32r,
           "f32": mybir.dt.float32}[dtype_mode]

    def mm(ap):
        return ap

    nc = bacc.Bacc("TRN2", target_bir_lowering=False, debug=False,
                   num_devices=NCORES)

    xt = nc.dram_tensor("xt", [D, N], MDT, kind="ExternalInput")
    wq = nc.dram_tensor("wq", [D, INNER], MDT, kind="ExternalInput")
    wkv = nc.dram_tensor("wkv", [D, 2 * INNER], MDT, kind="ExternalInput")
    wo = nc.dram_tensor("wo", [INNER, D], MDT, kind="ExternalInput")
    bo = nc.dram_tensor("bo", [1, D], MDT, kind="ExternalInput")
    cosk = nc.dram_tensor("cosk", [128, N], MDT, kind="ExternalInput")
    sink = nc.dram_tensor("sink", [128, N], MDT, kind="ExternalInput")
    sel = nc.dram_tensor("sel", [2, 128], MDT, kind="ExternalInput")
    ones1 = nc.dram_tensor("ones1", [1, 128], MDT, kind="ExternalInput")
    out = nc.dram_tensor("out", [R, D], F32, kind="ExternalOutput")

    KC = D // 128  # 8 contraction chunks of 128
    VW = HEADS * (DH + 1)  # 1040: v columns with a ones column per head

    with TileContext(nc) as tc:
        with tc.tile_pool(name="persist", bufs=1) as persist, \
             tc.tile_pool(name="wpool", bufs=3) as wpool, \
             tc.tile_pool(name="wo_pool", bufs=2) as wo_pool, \
             tc.tile_pool(name="pt", bufs=3) as pt_pool, \
             tc.tile_pool(name="rot_tmp", bufs=2) as rot_tmp, \
             tc.tile_pool(name="stg", bufs=2) as stg_pool, \
             tc.tile_pool(name="drp", bufs=2) as dr_pool:

            qt_sb = persist.tile([128, KC, R], MDT)          # qT (rotated)
            kt_sb = persist.tile([128, KC, N], MDT)          # kT (rotated)
            v_sb = persist.tile([128, KC, VW], MDT)          # v + ones cols
            at_sb = persist.tile([128, KC, R], MDT)          # A^T (attn out)
            xt_sb = persist.tile([128, KC, N], MDT)
            cos_sb = persist.tile([128, N], MDT)
            sin_sb = persist.tile([128, N], MDT)
            sel_sb = persist.tile([1, 2, 128], MDT)
            bo_sb = persist.tile([1, D], MDT)
            ones1_sb = persist.tile([1, 128], MDT)

            # ones columns of v (column DH of each head's 65-wide group)
            vv = v_sb.rearrange("p c (h e) -> p c h e", e=DH + 1)
            ones_col = vv[:, :, :, DH:DH + 1]
            if MDT == mybir.dt.float32r:
                ones_col = ones_col.bitcast(F32)
            nc.vector.memset(ones_col, 1.0)

            # ---------------- DMA emission ------------------------------
            # Direct (engine-queue) DMAs run ~2.5x the HWDGE ring rate, so
            # the big early tensors go on the queues of engines that are
            # idle at kernel start; the slow ring carries what is needed
            # late (trig at ~t25, wo at the very end, small tensors).
            wq_sb = wpool.tile([128, KC, INNER], MDT, tag="w", name="wq")
            wk_sb = wpool.tile([128, KC, INNER], MDT, tag="w", name="wk")
            wv_sb = wpool.tile([128, KC, INNER], MDT, tag="w", name="wv")
            # Queue slots are paced (~1.2us each regardless of size), so
            # use FEW, BIG transfers in consumption order.
            xt_r = xt.rearrange("(c p) m -> p c m", p=128)
            wq_r = wq.rearrange("(c p) m -> p c m", p=128)
            wkv_r = wkv.rearrange("(c p) m -> p c m", p=128)
            # gpsimd queue: wq k-pairs (feed q immediately), wk/xt-other
            # interleaved (both needed by kT(0) at ~t36), wv (needed by
            # the v blocks from ~t44), wo (end)
            for kp in range(0, KC, 2):
                nc.gpsimd.dma_start(out=wq_sb[:, kp:kp + 2, :],
                                    in_=wq_r[:, kp:kp + 2, :])
            for kp in range(0, KC, 4):
                nc.gpsimd.dma_start(out=wk_sb[:, kp:kp + 4, :],
                                    in_=wkv_r[:, kp:kp + 4, 0:INNER])
                nc.gpsimd.dma_start(out=xt_sb[:, kp:kp + 4, R:N],
                                    in_=xt_r[:, kp:kp + 4, R:N])
            for kp in range(0, KC, 4):
                nc.gpsimd.dma_start(out=wv_sb[:, kp:kp + 4, :],
                                    in_=wkv_r[:, kp:kp + 4, INNER:2 * INNER])
            wo_tiles = []
            for n in range(2):
                wo_sb = wo_pool.tile([128, KC, 512], MDT,
                                     name=f"wo{n}", tag="woh")
                nc.gpsimd.dma_start(
                    out=wo_sb[:],
                    in_=wo[:, n * 512:(n + 1) * 512].rearrange(
                        "(c p) m -> p c m", p=128))
                wo_tiles.append(wo_sb)
            # scalar queue: xt own half (feeds q) + trig only, so the
            # scalar engine's psum->sbuf copies are never queue-blocked
            for kp in range(0, KC, 4):
                nc.scalar.dma_start(out=xt_sb[:, kp:kp + 4, 0:R],
                                    in_=xt_r[:, kp:kp + 4, 0:R])
            nc.scalar.dma_start(out=cos_sb[:, :], in_=cosk[:, :])
            nc.scalar.dma_start(out=sin_sb[:, :], in_=sink[:, :])
            # sync ring: small attention-phase tensors
            nc.sync.dma_start(out=sel_sb[0:1, :, :],
                              in_=sel[:, :].unsqueeze(0))
            nc.sync.dma_start(out=bo_sb[:], in_=bo[:, :])
            nc.sync.dma_start(out=ones1_sb[:], in_=ones1[:, :])

            # ---------------- rotary helper (DVE) -----------------------
            # swap adjacent partitions (2i <-> 2i+1) per 32-lane group;
            # the rotate-half sign lives in the host-signed sine input
            SWAP_MASK = [i ^ 1 for i in range(32)]
            rot_n = [0]

            def rotary(dst, cos_slc, sin_slc):
                rot_n[0] += 1
                rt = rot_tmp.tile([128, 512], MDT,
                                  name=f"rt{rot_n[0]}", tag="rt")
                nc.vector.stream_shuffle(rt[:], dst, mask=SWAP_MASK)
                nc.vector.tensor_mul(rt[:], rt[:], sin_slc)
                nc.vector.tensor_mul(dst, dst, cos_slc)
                nc.vector.tensor_add(dst, dst, rt[:])

            # ---------------- q projection (k-major, 8 banks) -----------
            # two k-major passes of 4 output chunks each: the first pass's
            # banks are copied out mid-q, so the main loop's psum pools
            # (which reuse these banks) can start without waiting
            with tc.tile_pool(name="ps_q", bufs=8, space="PSUM") as ps_q:
                qps = [ps_q.tile([128, 512], F32, name=f"q{c}", tag="q")
                       for c in range(KC)]
                for half in range(2):
                    # 5+3 split: the main loop's psum pools reuse the
                    # first banks, whose copies retire earliest
                    cs = range(0, 5) if half == 0 else range(5, 8)
                    for k in range(KC):
                        for c in cs:
                            nc.tensor.matmul(
                                qps[c][:],
                                mm(wq_sb[:, k, c * 128:(c + 1) * 128]),
                                mm(xt_sb[:, k, 0:R]),
                                start=(k == 0), stop=(k == KC - 1))
                    for c in cs:
                        nc.scalar.copy(out=qt_sb[:, c, :], in_=qps[c][:])
                        rotary(qt_sb[:, c, :], cos_sb[:, 0:R],
                               sin_sb[:, 0:R])

            # ---------------- main interleaved loop ---------------------
            with tc.tile_pool(name="ps_kv", bufs=4, space="PSUM") as ps_kv, \
                 tc.tile_pool(name="ps_s", bufs=2, space="PSUM") as ps_s:

                pts = {}
                stages = {}

                def emit_k(c):
                    for jh in range(2):
                        kp = ps_kv.tile([128, 512], F32, tag="kv",
                                        name=f"k{c}_{jh}")
                        for k in range(KC):
                            nc.tensor.matmul(
                                kp[:],
                                mm(wk_sb[:, k, c * 128:(c + 1) * 128]),
                                mm(xt_sb[:, k, jh * 512:(jh + 1) * 512]),
                                start=(k == 0), stop=(k == KC - 1))
                        dst = kt_sb[:, c, jh * 512:(jh + 1) * 512]
                        nc.scalar.copy(out=dst, in_=kp[:])
                        rotary(dst, cos_sb[:, jh * 512:(jh + 1) * 512],
                               sin_sb[:, jh * 512:(jh + 1) * 512])

                def emit_v(m, nh):
                    vp = ps_kv.tile([128, 512], F32, tag="kv",
                                    name=f"v{m}_{nh}")
                    for k in range(KC):
                        nc.tensor.matmul(
                            vp[:],
                            mm(xt_sb[:, k, m * 128:(m + 1) * 128]),
                            mm(wv_sb[:, k, nh * 512:(nh + 1) * 512]),
                            start=(k == 0), stop=(k == KC - 1))
                    dst = vv[:, m, nh * 8:(nh + 1) * 8, 0:DH]
                    nc.vector.tensor_copy(
                        dst, vp[:].rearrange("p (h e) -> p h e", e=DH))

                def emit_spair(p, js):
                    """Packed scores for head pair p at j-chunks js: the
                    even head contracts over array rows 0-63 and the odd
                    head over rows 64-127, so the two matmuls run
                    concurrently in different row groups. One exp covers
                    both heads' j-chunk."""
                    c = p
                    if p not in pts:
                        pts[p] = pt_pool.tile([128, KC, 2, 512], MDT,
                                              name=f"pt{p}", tag="pt")
                    pt = pts[p]
                    for j in js:
                        s_ps = ps_s.tile([128, 2, 512], F32, tag="s")
                        for par in range(2):
                            po = par * 64
                            nc.tensor.matmul(
                                s_ps[:, par, :],
                                mm(kt_sb[po:po + 64, c,
                                         j * 128:(j + 1) * 128]),
                                mm(qt_sb[po:po + 64, c, :]),
                                start=True, stop=True)
                        nc.scalar.activation(
                            out=pt[:, j, :, :], in_=s_ps[:],
                            func=mybir.ActivationFunctionType.Exp,
                            scale=SCALE)

                def emit_pv(h):
                    c = h // 2
                    po = (h % 2) * 64
                    pt = pts[h // 2] if h % 2 == 0 else pts.pop(h // 2)
                    pv_ps = ps_kv.tile([128, 512], F32, tag="kv",
                                       name=f"pv{h}")
                    for j in range(KC):
                        nc.tensor.matmul(
                            pv_ps[0:DH + 1, :],
                            mm(v_sb[:, j, h * (DH + 1):(h + 1) * (DH + 1)]),
                            mm(pt[:, j, h % 2, :]),
                            start=(j == 0), stop=(j == KC - 1))
                    nc.vector.tensor_copy(at_sb[po:po + 64, c, :],
                                          pv_ps[0:DH, :])
                    if h % 2 == 0:
                        stages[c] = stg_pool.tile([1, 4, R], F32,
                                                  name=f"stg{c}", tag="stg")
                    # denom row: psum partition 64 -> partition 0 staging
                    # (reciprocal_approx_fast requires base partition 0)
                    nc.vector.tensor_copy(stages[c][0:1, h % 2, :],
                                          pv_ps[DH:DH + 1, :])

                def emit_norm(c):
                    # one fast-approx reciprocal over the pair's denoms,
                    # cast to matmul dtype, broadcast each head's 1/denom
                    # over its 64 partitions via two accumulating K=1
                    # selector matmuls, normalize chunk c of A^T
                    stage = stages.pop(c)
                    nc.vector.reciprocal_approx_fast(
                        stage[0:1, 2:4, :].rearrange("p a b -> p (a b)"),
                        stage[0:1, 0:2, :].rearrange("p a b -> p (a b)"))
                    drb = dr_pool.tile([1, 2, R], MDT,
                                       name=f"dr{c}", tag="dr")
                    nc.vector.tensor_copy(drb[0:1, :, :],
                                          stage[0:1, 2:4, :])
                    b_ps = ps_kv.tile([128, 512], F32, tag="kv",
                                      name=f"b{c}")
                    nc.tensor.matmul(
                        b_ps[:], mm(sel_sb[0:1, 0, :]), mm(drb[0:1, 0, :]),
                        start=True, stop=False)
                    nc.tensor.matmul(
                        b_ps[:], mm(sel_sb[0:1, 1, :]), mm(drb[0:1, 1, :]),
                        start=False, stop=True)
                    nc.vector.tensor_mul(at_sb[:, c, :],
                                         at_sb[:, c, :], b_ps[:])

                # schedule: scores lag k by 1 chunk, pv lags scores by 1,
                # norm lags pv by 1; v-blocks fill the early iterations
                # schedule: scores pair lags k by 1 chunk, pv lags scores
                # by 1, norm lags pv by 1; score j-rounds interleave with
                # pv/v blocks so the exp stream hides behind other PE work
                for c in range(KC):
                    emit_k(c)
                    p, q2 = c - 1, c - 2
                    if p >= 0:
                        emit_spair(p, [0, 1])
                    if q2 >= 0:
                        emit_pv(2 * q2)
                    if p >= 0:
                        emit_spair(p, [2, 3])
                    if q2 >= 0:
                        emit_pv(2 * q2 + 1)
                    if p >= 0:
                        emit_spair(p, [4, 5])
                    if c < 4:
                        nh, mg = divmod(c, 2)
                        for m in range(4 * mg, 4 * mg + 4):
                            emit_v(m, nh)
                    if c == 7:
                        # pull pair 7's first score rounds forward so its
                        # exp stream overlaps the last iteration
                        emit_spair(7, [0, 1])
                    if p >= 0:
                        emit_spair(p, [6, 7])
                    if c >= 3:
                        emit_norm(c - 3)
                # tail
                emit_pv(12)
                emit_spair(7, [2, 3])
                emit_pv(13)
                emit_spair(7, [4, 5])
                emit_norm(5)
                emit_spair(7, [6, 7])
                emit_pv(14)
                emit_pv(15)
                emit_norm(6)
                emit_norm(7)

            # ---------------- output projection -------------------------
            # m-outer, both n-halves per stationary at-block (each
            # stationary loads once and runs two matmuls back to back)
            with tc.tile_pool(name="ps_f", bufs=4, space="PSUM") as ps_f:
                fps = {}
                for m in range(4):
                    for n in range(2):
                        fps[n] = ps_f.tile([128, 512], F32, tag="f",
                                           name=f"f{m}_{n}")
                    for k in range(KC):
                        for n in range(2):
                            nc.tensor.matmul(
                                fps[n][:],
                                mm(at_sb[:, k, m * 128:(m + 1) * 128]),
                                mm(wo_tiles[n][:, k, :]),
                                start=(k == 0),
                                stop=(not has_bias and k == KC - 1))
                    for n in range(2):
                        if has_bias:
                            nc.tensor.matmul(
                                fps[n][:], mm(ones1_sb[:]),
                                mm(bo_sb[0:1, n * 512:(n + 1) * 512]),
                                start=False, stop=True)
                        # stage in the (now dead) weight pool buffers;
                        # alternate copy engines; store via the idle, fast
                        # gpsimd direct queue
                        o_sb = wpool.tile([128, 512], F32, tag="w",
                                          name=f"o{n}_{m}")
                        if n % 2 == 0:
                            nc.scalar.copy(out=o_sb[:], in_=fps[n][:])
                        else:
                            nc.vector.tensor_copy(o_sb[:], fps[n][:])
                        nc.gpsimd.dma_start(
                            out=out[m * 128:(m + 1) * 128,
                                    n * 512:(n + 1) * 512],
                            in_=o_sb[:])

    nc.compile()
    return nc


def _host_prep(x, rotary_emb, Wq, Wkv, Wo, bo, dtype_mode="f32"):
    """Build the per-core input maps."""
    if dtype_mode == "bf16":
        import ml_dtypes
        mnp = ml_dtypes.bfloat16
    else:
        mnp = np.float32
    x = np.asarray(x, dtype=np.float32)
    rotary_emb = np.asarray(rotary_emb, dtype=np.float32)
    Wq = np.ascontiguousarray(np.asarray(Wq, dtype=np.float32))
    Wkv = np.ascontiguousarray(np.asarray(Wkv, dtype=np.float32))
    Wo = np.ascontiguousarray(np.asarray(Wo, dtype=np.float32))
    bo_row = np.ascontiguousarray(np.asarray(bo, dtype=np.float32)[None, :])

    cosT = np.cos(rotary_emb).T.astype(np.float32)  # [64, 1024]
    sinT = np.sin(rotary_emb).T.astype(np.float32)
    cos2 = np.concatenate([cosT, cosT], axis=0)  # [128, n]
    sin2 = np.concatenate([sinT, sinT], axis=0)
    # rotate-half sign: rot[2i] = -x[2i+1], rot[2i+1] = +x[2i]; the device
    # only swaps lanes, so bake the sign into the sine rows
    sign = np.where(np.arange(128) % 2 == 0, -1.0, 1.0).astype(np.float32)
    sin2 = sin2 * sign[:, None]

    # selector rows: head-even -> partitions 0..63, head-odd -> 64..127
    sel = np.zeros((2, 128), dtype=np.float32)
    sel[0, 0:64] = 1.0
    sel[1, 64:128] = 1.0

    ones1 = np.ones((1, 128), dtype=np.float32)

    in_maps = []
    for core in range(NCORES):
        b, half = divmod(core, 2)
        perm = np.concatenate([
            np.arange(half * R, (half + 1) * R),
            np.arange((1 - half) * R, (2 - half) * R)])
        xt = np.ascontiguousarray(x[b].T[:, perm])  # [D, N] own half first
        in_maps.append({
            "xt": xt.astype(mnp),
            "wq": Wq.astype(mnp),
            "wkv": Wkv.astype(mnp),
            "wo": Wo.astype(mnp),
            "bo": bo_row.astype(mnp),
            "cosk": np.ascontiguousarray(cos2[:, perm]).astype(mnp),
            "sink": np.ascontiguousarray(sin2[:, perm]).astype(mnp),
            "sel": sel.astype(mnp),
            "ones1": ones1.astype(mnp),
        })
    return in_maps


def _run(inputs, trace=False, trace_cores=None):
    from concourse.bass_utils import run_bass_kernel_spmd

    has_bias = bool(np.any(np.asarray(inputs["bo"])))
    key = ("nc", DTYPE_MODE, has_bias)
    if key not in _CACHE:
        _CACHE[key] = _build(DTYPE_MODE, has_bias=has_bias)
    nc = _CACHE[key]

    in_maps = _host_prep(dtype_mode=DTYPE_MODE, **inputs)
    res = run_bass_kernel_spmd(nc, in_maps, list(range(NCORES)),
                               trace=trace, trace_cores=trace_cores)
    out = np.empty((B, N, D), dtype=np.float32)
    for core in range(NCORES):
        b, half = divmod(core, 2)
        out[b, half * R:(half + 1) * R, :] = res.results[core]["out"]
    return out, res


def kernel(**inputs):
    out, _ = _run(inputs, trace=False)
    return out

